# revision 11
# baseline (speedup 1.0000x reference)
"""Trainium2 Bass kernel for the GNN-VAE (GCNConv -> mean/max pool -> VAE MLPs).

Strategy (8 NeuronCores, SPMD):
  - Partition the 512 graphs into 8 groups of 64; the sorted `batch` vector
    makes each group a contiguous slab of nodes (and, after sorting edges by
    destination, a contiguous slab of edges).
  - Phase A  (per core): xw = x_slab @ Wg on the tensor engine, scaled by
    dinv -> u_slab (bf16).
  - AllGather u_slab across the 8 cores -> replicated u table (the gather
    source for message passing).
  - Phase C  (per core): per 128-node window, per 128-edge tile one indirect
    DMA gathers the messages u[src] (one row per partition).  One-hot
    matrices built on the vector engine turn the segment-sum into
    PSUM-accumulated matmuls; self-loops use a contiguous DMA + identity
    matmul.  Epilogue applies dinv[dst] + ReLU -> node_x.  Mean/count pooling
    accumulates into a persistent PSUM tile via one-hot(batch) matmuls.
  - Phase D  (per core): max pooling via one block-indirect gather (each
    graph's nodes are contiguous rows of node_x), a validity mask, and a
    segmented reduce_max.
  - Phase E  (per core): the tiny encoder/decoder MLPs on 64 graphs.
Host-side work is limited to index plumbing: sorting/partitioning edges,
degree counts, building gather index tables, and slicing inputs per core.
"""

import math
import numpy as np
import ml_dtypes

C = 8
FIN = 512
FG = 128
HID = 64
ZD = 64

BF16 = ml_dtypes.bfloat16

# module-level knobs (test.py pokes these)
DEBUG_DUMP = False
TRACE = False
TRACE_KWARGS = {}
LAST_RESULTS = None


def _preprocess(inputs):
    x = np.ascontiguousarray(np.asarray(inputs["x"], dtype=np.float32))
    ei = np.asarray(inputs["edge_index"]).astype(np.int64)
    batch = np.asarray(inputs["batch"]).astype(np.int64)
    eps = np.asarray(inputs["eps"], dtype=np.float32)

    N = x.shape[0]
    G = eps.shape[0]
    GPC = G // C
    E = ei.shape[1]

    sg = np.searchsorted(batch, np.arange(G + 1))
    core_bounds = sg[::GPC].copy()
    assert core_bounds.shape[0] == C + 1 and core_bounds[-1] == N
    ncs = np.diff(core_bounds)
    NPC = int(math.ceil(ncs.max() / 128) * 128)
    W = NPC // 128

    # degrees include the self-loop
    deg = (np.bincount(ei[1], minlength=N) + 1).astype(np.float32)
    dinv = (1.0 / np.sqrt(deg)).astype(np.float32)

    node_core = np.searchsorted(core_bounds, np.arange(N), side="right") - 1
    pid = (node_core * NPC + (np.arange(N) - core_bounds[node_core])).astype(np.int64)

    # real edges only, sorted by destination (self-loops handled separately)
    order = np.argsort(ei[1], kind="stable")
    dsts = ei[1][order]
    srcs_pid = pid[ei[0][order]].astype(np.int32)
    core_edge_bounds = np.searchsorted(dsts, core_bounds)

    dst_core_all = np.searchsorted(core_bounds, dsts, side="right") - 1
    dst_loc_all = dsts - core_bounds[dst_core_all]
    cw = dst_core_all * W + (dst_loc_all >> 7)
    cnts = np.bincount(cw, minlength=C * W)
    S = int(math.ceil(cnts.max() / 128))

    gsz = np.diff(sg)
    Lmax = int(gsz.max())
    SD = int(math.ceil(max(Lmax, 1) / 128) * 128)

    idx_arr = np.full((C, W, 128, S), 1 << 29, dtype=np.int32)
    dslot_arr = np.full((C, W, 128, S), -1.0, dtype=np.float32)
    blocal_arr = np.full((C, W * 128), -1.0, dtype=np.float32)
    dinv_arr = np.zeros((C, W * 128), dtype=np.float32)
    xt_arr = np.zeros((C, W, 128, FIN // 128, 128), dtype=np.float32)
    gstart_arr = np.zeros((C, GPC, 1), dtype=np.int32)
    dmask_arr = np.zeros((C, GPC, SD), dtype=np.float32)
    eps_arr = np.zeros((C, GPC, ZD), dtype=np.float32)

    for c in range(C):
        lo, hi = core_bounds[c], core_bounds[c + 1]
        n_c = hi - lo
        e0, e1 = core_edge_bounds[c], core_edge_bounds[c + 1]
        dloc = (dsts[e0:e1] - lo).astype(np.int64)
        spid = srcs_pid[e0:e1]
        win = dloc >> 7
        slot = (dloc & 127).astype(np.float32)
        starts = np.searchsorted(win, np.arange(W))
        rank = np.arange(e1 - e0) - starts[win]
        p = rank % 128
        j = rank // 128
        idx_arr[c, win, p, j] = spid
        dslot_arr[c, win, p, j] = slot

        blocal_arr[c, :n_c] = batch[lo:hi] - c * GPC
        dinv_arr[c, :n_c] = dinv[lo:hi]

        xs = np.zeros((NPC, FIN), dtype=np.float32)
        xs[:n_c] = x[lo:hi]
        # xt[w, p, k, m] = xs[w*128 + m, k*128 + p]
        xt_arr[c] = xs.reshape(W, 128, FIN // 128, 128).transpose(0, 3, 2, 1)

        for g in range(GPC):
            s = sg[c * GPC + g] - lo
            L = gsz[c * GPC + g]
            gstart_arr[c, g, 0] = s
            dmask_arr[c, g, :L] = 1.0
        eps_arr[c] = eps[c * GPC : (c + 1) * GPC]

    iota128 = np.tile(np.arange(128, dtype=np.float32), (128, 1)).astype(BF16)
    iotaG = np.tile(np.arange(GPC, dtype=np.float32), (128, 1)).astype(BF16)
    ones_col = np.ones((128, 1), dtype=BF16)
    ident64 = np.eye(64, dtype=np.float32)
    ident128b = np.eye(128, dtype=np.float32).astype(BF16)

    weights = {}
    for nm in ("Wg", "We1", "We2", "We3", "Wd1", "Wd2", "Wd3"):
        weights[nm] = np.ascontiguousarray(np.asarray(inputs[nm], dtype=np.float32))
    biases = {}
    for nm in ("bg", "be1", "be2", "be3", "bd1", "bd2", "bd3"):
        biases[nm] = np.asarray(inputs[nm], dtype=np.float32).reshape(1, -1)
    has_bias = {nm: bool(np.any(b != 0.0)) for nm, b in biases.items()}

    meta = dict(N=N, G=G, GPC=GPC, E=E, NPC=NPC, W=W, S=S, SD=SD, has_bias=has_bias)

    in_maps = []
    for c in range(C):
        m = dict(
            xt=xt_arr[c],
            idx=idx_arr[c],
            dslot=dslot_arr[c],
            blocal=blocal_arr[c].reshape(W, 128, 1),
            dinvw=dinv_arr[c].reshape(W, 128, 1),
            gstart=gstart_arr[c],
            dmask=dmask_arr[c].astype(BF16),
            eps_s=eps_arr[c],
            iota128=iota128,
            iotaG=iotaG,
            ones_col=ones_col,
            ident64=ident64,
            ident128b=ident128b,
        )
        for nm, wv in weights.items():
            m[nm] = wv
        for nm, bv in biases.items():
            if has_bias[nm]:
                m[nm] = bv
        in_maps.append(m)
    return meta, in_maps


_BUILD_CACHE = {}


def _build(meta):
    key = (meta["NPC"], meta["S"], meta["SD"], DEBUG_DUMP,
           tuple(sorted(meta["has_bias"].items())))
    if key in _BUILD_CACHE:
        return _BUILD_CACHE[key]

    from concourse import bass, bacc, tile, mybir
    from contextlib import ExitStack

    NPC, W, S, GPC = meta["NPC"], meta["W"], meta["S"], meta["GPC"]
    SD = meta["SD"]
    has_bias = meta["has_bias"]
    KC = FIN // 128  # k chunks for the input matmul

    f32 = mybir.dt.float32
    bf16 = mybir.dt.bfloat16
    i32 = mybir.dt.int32
    AF = mybir.ActivationFunctionType
    OP = mybir.AluOpType

    nc = bacc.Bacc(
        "TRN2",
        target_bir_lowering=False,
        debug=False,
        enable_asserts=False,
        num_devices=C,
    )

    # ---- I/O ----
    xt = nc.dram_tensor("xt", [W, 128, KC, 128], f32, kind="ExternalInput").ap()
    idx = nc.dram_tensor("idx", [W, 128, S], i32, kind="ExternalInput").ap()
    dslot = nc.dram_tensor("dslot", [W, 128, S], f32, kind="ExternalInput").ap()
    blocal = nc.dram_tensor("blocal", [W, 128, 1], f32, kind="ExternalInput").ap()
    dinvw = nc.dram_tensor("dinvw", [W, 128, 1], f32, kind="ExternalInput").ap()
    gstart = nc.dram_tensor("gstart", [GPC, 1], i32, kind="ExternalInput").ap()
    dmask = nc.dram_tensor("dmask", [GPC, SD], bf16, kind="ExternalInput").ap()
    eps_s = nc.dram_tensor("eps_s", [GPC, ZD], f32, kind="ExternalInput").ap()
    iota128 = nc.dram_tensor("iota128", [128, 128], bf16, kind="ExternalInput").ap()
    iotaG = nc.dram_tensor("iotaG", [128, GPC], bf16, kind="ExternalInput").ap()
    ones_col = nc.dram_tensor("ones_col", [128, 1], bf16, kind="ExternalInput").ap()
    ident64 = nc.dram_tensor("ident64", [64, 64], f32, kind="ExternalInput").ap()
    ident128b = nc.dram_tensor("ident128b", [128, 128], bf16, kind="ExternalInput").ap()
    wg = nc.dram_tensor("Wg", [FIN, FG], f32, kind="ExternalInput").ap()
    we1 = nc.dram_tensor("We1", [2 * FG, HID], f32, kind="ExternalInput").ap()
    we2 = nc.dram_tensor("We2", [HID, HID], f32, kind="ExternalInput").ap()
    we3 = nc.dram_tensor("We3", [HID, 2 * ZD], f32, kind="ExternalInput").ap()
    wd1 = nc.dram_tensor("Wd1", [ZD, HID], f32, kind="ExternalInput").ap()
    wd2 = nc.dram_tensor("Wd2", [HID, HID], f32, kind="ExternalInput").ap()
    wd3 = nc.dram_tensor("Wd3", [HID, FG], f32, kind="ExternalInput").ap()
    bias_aps = {}
    bias_shapes = dict(bg=FG, be1=HID, be2=HID, be3=2 * ZD, bd1=HID, bd2=HID, bd3=FG)
    for nm, n in bias_shapes.items():
        if has_bias[nm]:
            bias_aps[nm] = nc.dram_tensor(nm, [1, n], f32, kind="ExternalInput").ap()

    if DEBUG_DUMP:
        d_uslab = nc.dram_tensor("d_uslab", [NPC, FG], bf16, kind="ExternalOutput").ap()
        d_ufull = nc.dram_tensor("d_ufull", [C * NPC, FG], bf16, kind="ExternalOutput").ap()
        d_nodex = nc.dram_tensor("d_nodex", [NPC + SD, FG], bf16, kind="ExternalOutput").ap()
        d_mean = nc.dram_tensor("d_mean", [GPC, FG], f32, kind="ExternalOutput").ap()
        d_max = nc.dram_tensor("d_max", [GPC, FG], f32, kind="ExternalOutput").ap()
    mu_out = nc.dram_tensor("mu", [GPC, ZD], f32, kind="ExternalOutput").ap()
    sd_out = nc.dram_tensor("stddev", [GPC, ZD], f32, kind="ExternalOutput").ap()
    y_out = nc.dram_tensor("y", [GPC, FG], f32, kind="ExternalOutput").ap()

    # ---- internal DRAM ----
    u_slab = nc.dram_tensor("u_slab", [NPC, FG], bf16).ap()
    u_full = nc.dram_tensor("u_full", [C * NPC, FG], bf16, addr_space="Shared").ap()
    node_x = nc.dram_tensor("node_x", [NPC + SD, FG], bf16).ap()

    with tile.TileContext(nc) as tc, ExitStack() as ctx:
        consts = ctx.enter_context(tc.tile_pool(name="consts", bufs=1))

        wg_sb = consts.tile([128, KC * FG], f32, tag="wg")
        nc.sync.dma_start(
            out=wg_sb[:].rearrange("p (k m) -> p k m", k=KC),
            in_=wg.rearrange("(k p) m -> p k m", p=128),
        )
        iota128_sb = consts.tile([128, 128], bf16, tag="iota128")
        nc.sync.dma_start(out=iota128_sb[:], in_=iota128[:])
        iotaG_sb = consts.tile([128, GPC], bf16, tag="iotaG")
        nc.sync.dma_start(out=iotaG_sb[:], in_=iotaG[:])
        ones_sb = consts.tile([128, 1], bf16, tag="ones")
        nc.sync.dma_start(out=ones_sb[:], in_=ones_col[:])
        ident_sb = consts.tile([64, 64], f32, tag="ident")
        nc.sync.dma_start(out=ident_sb[:], in_=ident64[:])
        identb_sb = consts.tile([128, 128], bf16, tag="identb")
        nc.sync.dma_start(out=identb_sb[:], in_=ident128b[:])

        # ---------- Phase A: u_slab = dinv * (x @ Wg) ----------
        with (
            tc.tile_pool(name="pa_sbuf", bufs=3) as pa,
            tc.tile_pool(name="pa_psum", bufs=2, space="PSUM") as pap,
        ):
            for w in range(W):
                xt_sb = pa.tile([128, KC * 128], f32, tag="xt")
                nc.sync.dma_start(
                    out=xt_sb[:], in_=xt[w].rearrange("p k m -> p (k m)")
                )
                dv = pa.tile([128, 1], f32, tag="dv")
                nc.sync.dma_start(out=dv[:], in_=dinvw[w])
                ps = pap.tile([128, FG], f32, tag="pa_ps", space="PSUM")
                for k in range(KC):
                    nc.tensor.matmul(
                        ps[:],
                        lhsT=xt_sb[:, k * 128 : (k + 1) * 128],
                        rhs=wg_sb[:, k * FG : (k + 1) * FG],
                        start=(k == 0),
                        stop=(k == KC - 1),
                    )
                u_sb = pa.tile([128, FG], bf16, tag="u")
                nc.scalar.activation(u_sb[:], ps[:], AF.Copy, scale=dv[:])
                nc.sync.dma_start(out=u_slab[w * 128 : (w + 1) * 128, :], in_=u_sb[:])

        # ---------- AllGather ----------
        nc.gpsimd.collective_compute(
            "AllGather",
            mybir.AluOpType.bypass,
            replica_groups=[list(range(C))],
            ins=[u_slab[:]],
            outs=[u_full[:]],
        )

        # ---------- Phase C: message passing + mean pooling ----------
        pool_ps_pool = ctx.enter_context(
            tc.tile_pool(name="pool_ps", bufs=1, space="PSUM")
        )
        pool_ps = pool_ps_pool.tile([GPC, FG], f32, tag="pool", space="PSUM")
        pool_cnt = pool_ps_pool.tile([GPC, 2], f32, tag="poolcnt", space="PSUM")
        mean_sb = consts.tile([GPC, FG], f32, tag="mean")
        with (
            tc.tile_pool(name="pc_sbuf", bufs=4) as pcs,
            tc.tile_pool(name="pc_meta", bufs=12) as pcm,
            tc.tile_pool(name="pc_psum", bufs=2, space="PSUM") as pcp,
        ):
            for w in range(W):
                idx_sb = pcm.tile([128, S], i32, tag="idx")
                nc.sync.dma_start(out=idx_sb[:], in_=idx[w])
                ds_sb = pcm.tile([128, S], f32, tag="ds")
                nc.sync.dma_start(out=ds_sb[:], in_=dslot[w])
                bl_sb = pcm.tile([128, 1], f32, tag="bl")
                nc.sync.dma_start(out=bl_sb[:], in_=blocal[w])
                dv_sb = pcm.tile([128, 1], f32, tag="dvc")
                nc.sync.dma_start(out=dv_sb[:], in_=dinvw[w])

                gat = pcs.tile([128, (S + 1) * FG], bf16, tag="gat")
                if w < 4:
                    nc.vector.memset(gat[:], 0.0)
                for j in range(S):
                    nc.gpsimd.indirect_dma_start(
                        out=gat[:, j * FG : (j + 1) * FG],
                        out_offset=None,
                        in_=u_full[:],
                        in_offset=bass.IndirectOffsetOnAxis(
                            ap=idx_sb[:, j : j + 1], axis=0
                        ),
                        bounds_check=C * NPC - 1,
                        oob_is_err=False,
                    )
                # self-loop messages: own window's u rows, contiguous
                nc.sync.dma_start(
                    out=gat[:, S * FG : (S + 1) * FG],
                    in_=u_slab[w * 128 : (w + 1) * 128, :],
                )

                oh = pcs.tile([128, S * 128], bf16, tag="oh")
                for j in range(S):
                    nc.vector.tensor_scalar(
                        out=oh[:, j * 128 : (j + 1) * 128],
                        in0=iota128_sb[:],
                        scalar1=ds_sb[:, j : j + 1],
                        scalar2=None,
                        op0=OP.is_equal,
                    )

                wps = pcp.tile([128, FG], f32, tag="wps", space="PSUM")
                nmm = S + 1 + (1 if has_bias["bg"] else 0)
                for j in range(S):
                    nc.tensor.matmul(
                        wps[:],
                        lhsT=oh[:, j * 128 : (j + 1) * 128],
                        rhs=gat[:, j * FG : (j + 1) * FG],
                        start=(j == 0),
                        stop=False,
                    )
                nc.tensor.matmul(
                    wps[:],
                    lhsT=identb_sb[:],
                    rhs=gat[:, S * FG : (S + 1) * FG],
                    start=False,
                    stop=(nmm == S + 1),
                )
                if has_bias["bg"]:
                    bg_sb = pcm.tile([1, FG], f32, tag="bgrow")
                    nc.sync.dma_start(out=bg_sb[:], in_=bias_aps["bg"][:])
                    one_row = pcm.tile([1, 128], f32, tag="onerow")
                    nc.vector.memset(one_row[:], 1.0)
                    nc.tensor.matmul(
                        wps[:], lhsT=one_row[:], rhs=bg_sb[:], start=False, stop=True
                    )

                nx = pcs.tile([128, FG], bf16, tag="nx")
                nc.scalar.activation(nx[:], wps[:], AF.Relu, scale=dv_sb[:])

                boh = pcs.tile([128, GPC], bf16, tag="boh")
                nc.vector.tensor_scalar(
                    out=boh[:],
                    in0=iotaG_sb[:],
                    scalar1=bl_sb[:],
                    scalar2=None,
                    op0=OP.is_equal,
                )
                nc.tensor.matmul(
                    pool_ps[:, :FG],
                    lhsT=boh[:],
                    rhs=nx[:],
                    start=(w == 0),
                    stop=(w == W - 1),
                )
                nc.tensor.matmul(
                    pool_cnt[:, 0:1],
                    lhsT=boh[:],
                    rhs=ones_sb[:],
                    start=(w == 0),
                    stop=(w == W - 1),
                )
                nc.sync.dma_start(
                    out=node_x[w * 128 : (w + 1) * 128, :], in_=nx[:]
                )

            # zero tail rows (block-gather spillover for the last graphs)
            zt = pcs.tile([128, FG], bf16, tag="zt")
            nc.vector.memset(zt[:], 0.0)
            for t in range(SD // 128):
                nc.sync.dma_start(
                    out=node_x[NPC + t * 128 : NPC + (t + 1) * 128, :], in_=zt[:]
                )

            # mean = pool_sum / max(cnt, 1)
            cnt_sb = pcs.tile([GPC, 1], f32, tag="cnt")
            nc.vector.tensor_scalar(
                out=cnt_sb[:],
                in0=pool_cnt[:, 0:1],
                scalar1=1.0,
                scalar2=None,
                op0=OP.max,
            )
            rec_sb = pcs.tile([GPC, 1], f32, tag="rec")
            nc.vector.reciprocal(rec_sb[:], cnt_sb[:])
            nc.vector.tensor_scalar(
                out=mean_sb[:],
                in0=pool_ps[:, :FG],
                scalar1=rec_sb[:],
                scalar2=None,
                op0=OP.mult,
            )

        # ---------- Phase D: max pooling ----------
        # Each graph's nodes are contiguous rows of node_x; the block-gather
        # reads SD rows per graph starting at gstart[g], the mask zeroes
        # rows belonging to the next graphs (node_x >= 0 so 0 is neutral).
        max_sb = consts.tile([GPC, FG], f32, tag="maxp")
        with tc.tile_pool(name="pd_sbuf", bufs=1) as pd:
            gs_sb = pd.tile([GPC, 1], i32, tag="gs")
            nc.sync.dma_start(out=gs_sb[:], in_=gstart[:])
            dm_sb = pd.tile([GPC, SD], bf16, tag="dm")
            nc.sync.dma_start(out=dm_sb[:], in_=dmask[:])
            gat_d = pd.tile([GPC, SD * FG], bf16, tag="gat_d")
            nc.gpsimd.indirect_dma_start(
                out=gat_d[:],
                out_offset=None,
                in_=node_x[:],
                in_offset=bass.IndirectOffsetOnAxis(ap=gs_sb[:, 0:1], axis=0),
            )
            nc.vector.tensor_tensor(
                out=gat_d[:].rearrange("g (s f) -> g s f", f=FG),
                in0=gat_d[:].rearrange("g (s f) -> g s f", f=FG),
                in1=dm_sb[:, :, None].to_broadcast([GPC, SD, FG]),
                op=OP.mult,
            )
            nc.vector.reduce_max(
                out=max_sb[:],
                in_=gat_d[:].rearrange("g (s f) -> g f s", f=FG),
                axis=mybir.AxisListType.X,
            )

        # ---------- Phase E: MLP head ----------
        def linear(ctx_pool, psum_pool, act_sb, w_ap, kdim, ndim, bias_nm):
            """act_sb: [GPC, kdim] f32 sbuf -> psum tile [GPC, ndim] f32."""
            nch = (kdim + 127) // 128
            ps = psum_pool.tile([GPC, max(ndim, 2)], f32, tag="mlp_ps", space="PSUM")
            w_sb = ctx_pool.tile([128, nch * ndim], f32, tag="mlp_w")
            if nch == 1:
                nc.sync.dma_start(out=w_sb[:kdim, :ndim], in_=w_ap[:])
            else:
                nc.sync.dma_start(
                    out=w_sb[:].rearrange("p (k m) -> p k m", k=nch),
                    in_=w_ap.rearrange("(k p) m -> p k m", p=128),
                )
            nmm = nch + (1 if has_bias[bias_nm] else 0)
            for ki in range(nch):
                klo = ki * 128
                kk = min(128, kdim - klo)
                tps = psum_pool.tile([128, GPC], f32, tag="tr_ps", space="PSUM")
                nc.tensor.transpose(
                    out=tps[:kk, :GPC],
                    in_=act_sb[:, klo : klo + kk],
                    identity=ident_sb[:GPC, :GPC],
                )
                at_sb = ctx_pool.tile([128, GPC], f32, tag="at")
                nc.vector.tensor_copy(at_sb[:kk, :], tps[:kk, :])
                nc.tensor.matmul(
                    ps[:, :ndim],
                    lhsT=at_sb[:kk, :GPC],
                    rhs=w_sb[:kk, ki * ndim : (ki + 1) * ndim],
                    start=(ki == 0),
                    stop=(ki == nmm - 1),
                )
            if has_bias[bias_nm]:
                b_sb = ctx_pool.tile([1, max(ndim, 2)], f32, tag="mlp_b")
                nc.sync.dma_start(out=b_sb[:, :ndim], in_=bias_aps[bias_nm][:])
                one_row = ctx_pool.tile([1, GPC], f32, tag="onerow_e")
                nc.vector.memset(one_row[:], 1.0)
                nc.tensor.matmul(
                    ps[:, :ndim], lhsT=one_row[:], rhs=b_sb[:, :ndim],
                    start=False, stop=True,
                )
            return ps

        def elu(pe, out_sb, in_ps, n):
            r = pe.tile([GPC, n], f32, tag="elu_r")
            nc.scalar.activation(r[:], in_ps[:, :n], AF.Relu)
            m = pe.tile([GPC, n], f32, tag="elu_m")
            nc.vector.tensor_tensor(out=m[:], in0=in_ps[:, :n], in1=r[:], op=OP.subtract)
            e = pe.tile([GPC, n], f32, tag="elu_e")
            nc.scalar.activation(e[:], m[:], AF.Exp)
            nc.vector.tensor_tensor(out=e[:], in0=e[:], in1=r[:], op=OP.add)
            nc.vector.tensor_scalar(
                out=out_sb[:, :n], in0=e[:], scalar1=1.0, scalar2=None, op0=OP.subtract
            )

        with (
            tc.tile_pool(name="pe_sbuf", bufs=2) as pe,
            tc.tile_pool(name="pe_psum", bufs=2, space="PSUM") as pep,
        ):
            gx = pe.tile([GPC, 2 * FG], f32, tag="gx")
            nc.vector.tensor_copy(gx[:, :FG], mean_sb[:])
            nc.vector.tensor_copy(gx[:, FG:], max_sb[:])

            h1ps = linear(pe, pep, gx, we1, 2 * FG, HID, "be1")
            h1 = pe.tile([GPC, HID], f32, tag="h1")
            elu(pe, h1, h1ps, HID)

            h2ps = linear(pe, pep, h1, we2, HID, HID, "be2")
            h2 = pe.tile([GPC, HID], f32, tag="h2")
            nc.scalar.activation(h2[:], h2ps[:, :HID], AF.Tanh)

            mlps = linear(pe, pep, h2, we3, HID, 2 * ZD, "be3")
            mu_sb = pe.tile([GPC, ZD], f32, tag="mu")
            nc.vector.tensor_copy(mu_sb[:], mlps[:, :ZD])
            # softplus(x) = ln(1 + exp(x)); |x| is small here so this is stable
            sp_e = pe.tile([GPC, ZD], f32, tag="sp_e")
            nc.scalar.activation(sp_e[:], mlps[:, ZD : 2 * ZD], AF.Exp)
            nc.vector.tensor_scalar(
                out=sp_e[:], in0=sp_e[:], scalar1=1.0, scalar2=None, op0=OP.add
            )
            sd_sb = pe.tile([GPC, ZD], f32, tag="sd")
            nc.scalar.activation(sd_sb[:], sp_e[:], AF.Ln)
            nc.vector.tensor_scalar(
                out=sd_sb[:], in0=sd_sb[:], scalar1=1e-6, scalar2=None, op0=OP.add
            )
            eps_sb = pe.tile([GPC, ZD], f32, tag="eps")
            nc.sync.dma_start(out=eps_sb[:], in_=eps_s[:])
            z = pe.tile([GPC, ZD], f32, tag="z")
            nc.vector.tensor_tensor(out=z[:], in0=eps_sb[:], in1=sd_sb[:], op=OP.mult)
            nc.vector.tensor_tensor(out=z[:], in0=z[:], in1=mu_sb[:], op=OP.add)

            d1ps = linear(pe, pep, z, wd1, ZD, HID, "bd1")
            d1 = pe.tile([GPC, HID], f32, tag="d1")
            nc.scalar.activation(d1[:], d1ps[:, :HID], AF.Tanh)

            d2ps = linear(pe, pep, d1, wd2, HID, HID, "bd2")
            d2 = pe.tile([GPC, HID], f32, tag="d2")
            elu(pe, d2, d2ps, HID)

            yps = linear(pe, pep, d2, wd3, HID, FG, "bd3")
            y_sb = pe.tile([GPC, FG], f32, tag="ysb")
            nc.scalar.activation(y_sb[:], yps[:, :FG], AF.Sigmoid)
            nc.vector.tensor_scalar(
                out=y_sb[:],
                in0=y_sb[:],
                scalar1=1e-8,
                scalar2=1.0 - 1e-8,
                op0=OP.max,
                op1=OP.min,
            )

            nc.sync.dma_start(out=mu_out[:], in_=mu_sb[:])
            nc.sync.dma_start(out=sd_out[:], in_=sd_sb[:])
            nc.sync.dma_start(out=y_out[:], in_=y_sb[:])

        if DEBUG_DUMP:
            nc.sync.dma_start(out=d_uslab[:], in_=u_slab[:])
            nc.sync.dma_start(out=d_ufull[:], in_=u_full[:])
            nc.sync.dma_start(out=d_nodex[:], in_=node_x[:])
            nc.sync.dma_start(out=d_mean[:], in_=mean_sb[:])
            nc.sync.dma_start(out=d_max[:], in_=max_sb[:])

    nc.compile()
    _BUILD_CACHE[key] = nc
    return nc


def _install_ntff_hook():
    """Provide antenv.axon_hooks (missing in this image) so that
    run_bass_kernel_spmd(trace=True) can capture NTFF profiles via the
    axon .so's C ABI."""
    import sys, types, ctypes, contextlib

    try:
        from antenv.axon_hooks import get_axon_ntff_profile_hook  # noqa: F401

        return
    except ImportError:
        pass
    so_path = "/opt/axon/libaxon_pjrt.so"
    try:
        lib = ctypes.CDLL(so_path)
        lib.axon_start_nrt_profile.argtypes = [
            ctypes.POINTER(ctypes.c_int64),
            ctypes.c_size_t,
        ]
        lib.axon_start_nrt_profile.restype = ctypes.c_int64
        lib.axon_stop_nrt_profile.argtypes = [ctypes.c_char_p]
        lib.axon_stop_nrt_profile.restype = ctypes.c_int64
    except (OSError, AttributeError):
        lib = None

    @contextlib.contextmanager
    def _hook(output_dir, device_ids):
        import jax

        jax.devices()
        if device_ids:
            ids = (ctypes.c_int64 * len(device_ids))(*device_ids)
            rc = lib.axon_start_nrt_profile(ids, len(device_ids))
        else:
            rc = lib.axon_start_nrt_profile(None, 0)
        if rc != 0:
            raise RuntimeError(f"axon_start_nrt_profile rc={rc}")
        try:
            yield
        finally:
            n = lib.axon_stop_nrt_profile(str(output_dir).encode())
            print(f"ntff profile: {n} file(s) written to {output_dir}")

    mod = types.ModuleType("antenv.axon_hooks")
    mod.get_axon_ntff_profile_hook = lambda: (_hook if lib is not None else None)
    mod.set_axon_ntff_profile_hook = lambda h: None
    sys.modules["antenv.axon_hooks"] = mod


def kernel(**inputs):
    global LAST_RESULTS
    from concourse import bass_utils

    if TRACE:
        _install_ntff_hook()

    meta, in_maps = _preprocess(inputs)
    nc = _build(meta)
    res = bass_utils.run_bass_kernel_spmd(
        nc,
        in_maps,
        core_ids=list(range(C)),
        trace=TRACE,
        **TRACE_KWARGS,
    )
    LAST_RESULTS = res
    mu = np.concatenate([res.results[c]["mu"] for c in range(C)], axis=0)
    sd = np.concatenate([res.results[c]["stddev"] for c in range(C)], axis=0)
    y = np.concatenate([res.results[c]["y"] for c in range(C)], axis=0)
    return mu, sd, y


# revision 13
# speedup vs baseline: 1.0446x; 1.0446x over previous
"""Trainium2 Bass kernel for the GNN-VAE (GCNConv -> mean/max pool -> VAE MLPs).

Strategy (8 NeuronCores, SPMD):
  - Partition the 512 graphs into 8 groups of 64; the sorted `batch` vector
    makes each group a contiguous slab of nodes (and, after sorting edges by
    destination, a contiguous slab of edges).
  - Phase A  (per core): xw = x_slab @ Wg on the tensor engine, scaled by
    dinv -> u_slab (bf16).
  - AllGather u_slab across the 8 cores -> replicated u table (the gather
    source for message passing).
  - Phase C  (per core): per 128-node window, per 128-edge tile one indirect
    DMA gathers the messages u[src] (one row per partition).  One-hot
    matrices built on the vector engine turn the segment-sum into
    PSUM-accumulated matmuls; self-loops use a contiguous DMA + identity
    matmul.  Epilogue applies dinv[dst] + ReLU -> node_x.  Mean/count pooling
    accumulates into a persistent PSUM tile via one-hot(batch) matmuls.
  - Phase D  (per core): max pooling via one block-indirect gather (each
    graph's nodes are contiguous rows of node_x), a validity mask, and a
    segmented reduce_max.
  - Phase E  (per core): the tiny encoder/decoder MLPs on 64 graphs.
Host-side work is limited to index plumbing: sorting/partitioning edges,
degree counts, building gather index tables, and slicing inputs per core.
"""

import math
import numpy as np
import ml_dtypes

C = 8
FIN = 512
FG = 128
HID = 64
ZD = 64

BF16 = ml_dtypes.bfloat16

# module-level knobs (test.py pokes these)
DEBUG_DUMP = False
TRACE = False
TRACE_KWARGS = {}
LAST_RESULTS = None


def _preprocess(inputs):
    x = np.ascontiguousarray(np.asarray(inputs["x"], dtype=np.float32))
    ei = np.asarray(inputs["edge_index"]).astype(np.int64)
    batch = np.asarray(inputs["batch"]).astype(np.int64)
    eps = np.asarray(inputs["eps"], dtype=np.float32)

    N = x.shape[0]
    G = eps.shape[0]
    GPC = G // C
    E = ei.shape[1]

    sg = np.searchsorted(batch, np.arange(G + 1))
    core_bounds = sg[::GPC].copy()
    assert core_bounds.shape[0] == C + 1 and core_bounds[-1] == N
    ncs = np.diff(core_bounds)
    NPC = int(math.ceil(ncs.max() / 128) * 128)
    W = NPC // 128

    # degrees include the self-loop
    deg = (np.bincount(ei[1], minlength=N) + 1).astype(np.float32)
    dinv = (1.0 / np.sqrt(deg)).astype(np.float32)

    node_core = np.searchsorted(core_bounds, np.arange(N), side="right") - 1
    pid = (node_core * NPC + (np.arange(N) - core_bounds[node_core])).astype(np.int64)

    # real edges only, sorted by destination (self-loops handled separately)
    order = np.argsort(ei[1], kind="stable")
    dsts = ei[1][order]
    srcs_pid = pid[ei[0][order]].astype(np.int32)
    core_edge_bounds = np.searchsorted(dsts, core_bounds)

    dst_core_all = np.searchsorted(core_bounds, dsts, side="right") - 1
    dst_loc_all = dsts - core_bounds[dst_core_all]
    cw = dst_core_all * W + (dst_loc_all >> 7)
    cnts = np.bincount(cw, minlength=C * W)
    S = int(math.ceil(cnts.max() / 128))

    gsz = np.diff(sg)
    Lmax = int(gsz.max())
    SD = int(math.ceil(max(Lmax, 1) / 128) * 128)

    idx_arr = np.zeros((C, W, 128, S), dtype=np.int32)
    dslot_arr = np.full((C, W, 128, S), -1.0, dtype=np.float32)
    blocal_arr = np.full((C, W * 128), -1.0, dtype=np.float32)
    dinv_arr = np.zeros((C, W * 128), dtype=np.float32)
    xt_arr = np.zeros((C, W, 128, FIN // 128, 128), dtype=np.float32)
    gstart_arr = np.zeros((C, GPC, 1), dtype=np.int32)
    dmask_arr = np.zeros((C, GPC, SD), dtype=np.float32)
    eps_arr = np.zeros((C, GPC, ZD), dtype=np.float32)

    for c in range(C):
        lo, hi = core_bounds[c], core_bounds[c + 1]
        n_c = hi - lo
        e0, e1 = core_edge_bounds[c], core_edge_bounds[c + 1]
        dloc = (dsts[e0:e1] - lo).astype(np.int64)
        spid = srcs_pid[e0:e1]
        win = dloc >> 7
        slot = (dloc & 127).astype(np.float32)
        starts = np.searchsorted(win, np.arange(W))
        rank = np.arange(e1 - e0) - starts[win]
        p = rank % 128
        j = rank // 128
        idx_arr[c, win, p, j] = spid
        dslot_arr[c, win, p, j] = slot

        blocal_arr[c, :n_c] = batch[lo:hi] - c * GPC
        dinv_arr[c, :n_c] = dinv[lo:hi]

        xs = np.zeros((NPC, FIN), dtype=np.float32)
        xs[:n_c] = x[lo:hi]
        # xt[w, p, k, m] = xs[w*128 + m, k*128 + p]
        xt_arr[c] = xs.reshape(W, 128, FIN // 128, 128).transpose(0, 3, 2, 1)

        for g in range(GPC):
            s = sg[c * GPC + g] - lo
            L = gsz[c * GPC + g]
            gstart_arr[c, g, 0] = s
            dmask_arr[c, g, :L] = 1.0
        eps_arr[c] = eps[c * GPC : (c + 1) * GPC]

    iota128 = np.tile(np.arange(128, dtype=np.float32), (128, 1)).astype(BF16)
    iotaG = np.tile(np.arange(GPC, dtype=np.float32), (128, 1)).astype(BF16)
    ones_col = np.ones((128, 1), dtype=BF16)
    ident64 = np.eye(64, dtype=np.float32)
    ident128b = np.eye(128, dtype=np.float32).astype(BF16)

    weights = {}
    for nm in ("Wg", "We1", "We2", "We3", "Wd1", "Wd2", "Wd3"):
        weights[nm] = np.ascontiguousarray(np.asarray(inputs[nm], dtype=np.float32))
    biases = {}
    for nm in ("bg", "be1", "be2", "be3", "bd1", "bd2", "bd3"):
        biases[nm] = np.asarray(inputs[nm], dtype=np.float32).reshape(1, -1)
    has_bias = {nm: bool(np.any(b != 0.0)) for nm, b in biases.items()}

    meta = dict(N=N, G=G, GPC=GPC, E=E, NPC=NPC, W=W, S=S, SD=SD, has_bias=has_bias)

    in_maps = []
    for c in range(C):
        m = dict(
            xt=xt_arr[c].astype(BF16),
            idx=idx_arr[c],
            dslot=dslot_arr[c],
            blocal=blocal_arr[c].reshape(W, 128, 1),
            dinvw=dinv_arr[c].reshape(W, 128, 1),
            gstart=gstart_arr[c],
            dmask=dmask_arr[c].astype(BF16),
            eps_s=eps_arr[c],
            iota128=iota128,
            iotaG=iotaG,
            ones_col=ones_col,
            ident64=ident64,
            ident128b=ident128b,
        )
        for nm, wv in weights.items():
            m[nm] = wv
        for nm, bv in biases.items():
            if has_bias[nm]:
                m[nm] = bv
        in_maps.append(m)
    return meta, in_maps


_BUILD_CACHE = {}


def _build(meta):
    key = (meta["NPC"], meta["S"], meta["SD"], DEBUG_DUMP,
           tuple(sorted(meta["has_bias"].items())))
    if key in _BUILD_CACHE:
        return _BUILD_CACHE[key]

    from concourse import bass, bacc, tile, mybir
    from contextlib import ExitStack

    NPC, W, S, GPC = meta["NPC"], meta["W"], meta["S"], meta["GPC"]
    SD = meta["SD"]
    has_bias = meta["has_bias"]
    KC = FIN // 128  # k chunks for the input matmul

    f32 = mybir.dt.float32
    bf16 = mybir.dt.bfloat16
    i32 = mybir.dt.int32
    AF = mybir.ActivationFunctionType
    OP = mybir.AluOpType

    nc = bacc.Bacc(
        "TRN2",
        target_bir_lowering=False,
        debug=False,
        enable_asserts=False,
        num_devices=C,
    )

    # ---- I/O ----
    xt = nc.dram_tensor("xt", [W, 128, KC, 128], bf16, kind="ExternalInput").ap()
    idx = nc.dram_tensor("idx", [W, 128, S], i32, kind="ExternalInput").ap()
    dslot = nc.dram_tensor("dslot", [W, 128, S], f32, kind="ExternalInput").ap()
    blocal = nc.dram_tensor("blocal", [W, 128, 1], f32, kind="ExternalInput").ap()
    dinvw = nc.dram_tensor("dinvw", [W, 128, 1], f32, kind="ExternalInput").ap()
    gstart = nc.dram_tensor("gstart", [GPC, 1], i32, kind="ExternalInput").ap()
    dmask = nc.dram_tensor("dmask", [GPC, SD], bf16, kind="ExternalInput").ap()
    eps_s = nc.dram_tensor("eps_s", [GPC, ZD], f32, kind="ExternalInput").ap()
    iota128 = nc.dram_tensor("iota128", [128, 128], bf16, kind="ExternalInput").ap()
    iotaG = nc.dram_tensor("iotaG", [128, GPC], bf16, kind="ExternalInput").ap()
    ones_col = nc.dram_tensor("ones_col", [128, 1], bf16, kind="ExternalInput").ap()
    ident64 = nc.dram_tensor("ident64", [64, 64], f32, kind="ExternalInput").ap()
    ident128b = nc.dram_tensor("ident128b", [128, 128], bf16, kind="ExternalInput").ap()
    wg = nc.dram_tensor("Wg", [FIN, FG], f32, kind="ExternalInput").ap()
    we1 = nc.dram_tensor("We1", [2 * FG, HID], f32, kind="ExternalInput").ap()
    we2 = nc.dram_tensor("We2", [HID, HID], f32, kind="ExternalInput").ap()
    we3 = nc.dram_tensor("We3", [HID, 2 * ZD], f32, kind="ExternalInput").ap()
    wd1 = nc.dram_tensor("Wd1", [ZD, HID], f32, kind="ExternalInput").ap()
    wd2 = nc.dram_tensor("Wd2", [HID, HID], f32, kind="ExternalInput").ap()
    wd3 = nc.dram_tensor("Wd3", [HID, FG], f32, kind="ExternalInput").ap()
    bias_aps = {}
    bias_shapes = dict(bg=FG, be1=HID, be2=HID, be3=2 * ZD, bd1=HID, bd2=HID, bd3=FG)
    for nm, n in bias_shapes.items():
        if has_bias[nm]:
            bias_aps[nm] = nc.dram_tensor(nm, [1, n], f32, kind="ExternalInput").ap()

    if DEBUG_DUMP:
        d_uslab = nc.dram_tensor("d_uslab", [NPC, FG], bf16, kind="ExternalOutput").ap()
        d_ufull = nc.dram_tensor("d_ufull", [C * NPC, FG], bf16, kind="ExternalOutput").ap()
        d_nodex = nc.dram_tensor("d_nodex", [NPC + SD, FG], bf16, kind="ExternalOutput").ap()
        d_mean = nc.dram_tensor("d_mean", [GPC, FG], f32, kind="ExternalOutput").ap()
        d_max = nc.dram_tensor("d_max", [GPC, FG], f32, kind="ExternalOutput").ap()
    mu_out = nc.dram_tensor("mu", [GPC, ZD], f32, kind="ExternalOutput").ap()
    sd_out = nc.dram_tensor("stddev", [GPC, ZD], f32, kind="ExternalOutput").ap()
    y_out = nc.dram_tensor("y", [GPC, FG], f32, kind="ExternalOutput").ap()

    # ---- internal DRAM ----
    u_slab = nc.dram_tensor("u_slab", [NPC, FG], bf16).ap()
    u_full = nc.dram_tensor("u_full", [C * NPC, FG], bf16, addr_space="Shared").ap()
    node_x = nc.dram_tensor("node_x", [NPC + SD, FG], bf16).ap()

    with tile.TileContext(nc) as tc, ExitStack() as ctx:
        consts = ctx.enter_context(tc.tile_pool(name="consts", bufs=1))

        wg_sb = consts.tile([128, KC * FG], bf16, tag="wg")
        nc.gpsimd.dma_start(
            out=wg_sb[:].rearrange("p (k m) -> p k m", k=KC),
            in_=wg.rearrange("(k p) m -> p k m", p=128),
        )
        iota128_sb = consts.tile([128, 128], bf16, tag="iota128")
        nc.sync.dma_start(out=iota128_sb[:], in_=iota128[:])
        iotaG_sb = consts.tile([128, GPC], bf16, tag="iotaG")
        nc.sync.dma_start(out=iotaG_sb[:], in_=iotaG[:])
        ones_sb = consts.tile([128, 1], bf16, tag="ones")
        nc.sync.dma_start(out=ones_sb[:], in_=ones_col[:])
        ident_sb = consts.tile([64, 64], f32, tag="ident")
        nc.sync.dma_start(out=ident_sb[:], in_=ident64[:])
        identb_sb = consts.tile([128, 128], bf16, tag="identb")
        nc.sync.dma_start(out=identb_sb[:], in_=ident128b[:])

        # ---------- Phase A: u_slab = dinv * (x @ Wg) ----------
        with (
            tc.tile_pool(name="pa_sbuf", bufs=3) as pa,
            tc.tile_pool(name="pa_psum", bufs=2, space="PSUM") as pap,
        ):
            for w in range(W):
                xt_sb = pa.tile([128, KC * 128], bf16, tag="xt")
                nc.sync.dma_start(
                    out=xt_sb[:], in_=xt[w].rearrange("p k m -> p (k m)")
                )
                dv = pa.tile([128, 1], f32, tag="dv")
                nc.sync.dma_start(out=dv[:], in_=dinvw[w])
                ps = pap.tile([128, FG], f32, tag="pa_ps", space="PSUM")
                for k in range(KC):
                    nc.tensor.matmul(
                        ps[:],
                        lhsT=xt_sb[:, k * 128 : (k + 1) * 128],
                        rhs=wg_sb[:, k * FG : (k + 1) * FG],
                        start=(k == 0),
                        stop=(k == KC - 1),
                    )
                u_sb = pa.tile([128, FG], bf16, tag="u")
                nc.scalar.activation(u_sb[:], ps[:], AF.Copy, scale=dv[:])
                nc.sync.dma_start(out=u_slab[w * 128 : (w + 1) * 128, :], in_=u_sb[:])

        # ---------- AllGather ----------
        nc.gpsimd.collective_compute(
            "AllGather",
            mybir.AluOpType.bypass,
            replica_groups=[list(range(C))],
            ins=[u_slab[:]],
            outs=[u_full[:]],
        )

        # ---------- Phase C: message passing + mean pooling ----------
        pool_ps_pool = ctx.enter_context(
            tc.tile_pool(name="pool_ps", bufs=1, space="PSUM")
        )
        pool_ps = pool_ps_pool.tile([GPC, FG], f32, tag="pool", space="PSUM")
        pool_cnt = pool_ps_pool.tile([GPC, 2], f32, tag="poolcnt", space="PSUM")
        mean_sb = consts.tile([GPC, FG], f32, tag="mean")
        with (
            tc.tile_pool(name="pc_sbuf", bufs=4) as pcs,
            tc.tile_pool(name="pc_meta", bufs=12) as pcm,
            tc.tile_pool(name="pc_psum", bufs=2, space="PSUM") as pcp,
        ):
            for w in range(W):
                idx_sb = pcm.tile([128, S], i32, tag="idx")
                nc.sync.dma_start(out=idx_sb[:], in_=idx[w])
                ds_sb = pcm.tile([128, S], f32, tag="ds")
                nc.sync.dma_start(out=ds_sb[:], in_=dslot[w])
                bl_sb = pcm.tile([128, 1], f32, tag="bl")
                nc.sync.dma_start(out=bl_sb[:], in_=blocal[w])
                dv_sb = pcm.tile([128, 1], f32, tag="dvc")
                nc.sync.dma_start(out=dv_sb[:], in_=dinvw[w])

                gat = pcs.tile([128, (S + 1) * FG], bf16, tag="gat")
                for j in range(S):
                    nc.gpsimd.indirect_dma_start(
                        out=gat[:, j * FG : (j + 1) * FG],
                        out_offset=None,
                        in_=u_full[:],
                        in_offset=bass.IndirectOffsetOnAxis(
                            ap=idx_sb[:, j : j + 1], axis=0
                        ),
                    )
                # self-loop messages: own window's u rows, contiguous
                nc.sync.dma_start(
                    out=gat[:, S * FG : (S + 1) * FG],
                    in_=u_slab[w * 128 : (w + 1) * 128, :],
                )

                oh = pcs.tile([128, S * 128], bf16, tag="oh")
                for j in range(S):
                    nc.vector.tensor_scalar(
                        out=oh[:, j * 128 : (j + 1) * 128],
                        in0=iota128_sb[:],
                        scalar1=ds_sb[:, j : j + 1],
                        scalar2=None,
                        op0=OP.is_equal,
                    )

                wps = pcp.tile([128, FG], f32, tag="wps", space="PSUM")
                nmm = S + 1 + (1 if has_bias["bg"] else 0)
                for j in range(S):
                    nc.tensor.matmul(
                        wps[:],
                        lhsT=oh[:, j * 128 : (j + 1) * 128],
                        rhs=gat[:, j * FG : (j + 1) * FG],
                        start=(j == 0),
                        stop=False,
                    )
                nc.tensor.matmul(
                    wps[:],
                    lhsT=identb_sb[:],
                    rhs=gat[:, S * FG : (S + 1) * FG],
                    start=False,
                    stop=(nmm == S + 1),
                )
                if has_bias["bg"]:
                    bg_sb = pcm.tile([1, FG], f32, tag="bgrow")
                    nc.sync.dma_start(out=bg_sb[:], in_=bias_aps["bg"][:])
                    one_row = pcm.tile([1, 128], f32, tag="onerow")
                    nc.vector.memset(one_row[:], 1.0)
                    nc.tensor.matmul(
                        wps[:], lhsT=one_row[:], rhs=bg_sb[:], start=False, stop=True
                    )

                nx = pcs.tile([128, FG], bf16, tag="nx")
                nc.scalar.activation(nx[:], wps[:], AF.Relu, scale=dv_sb[:])

                boh = pcs.tile([128, GPC], bf16, tag="boh")
                nc.vector.tensor_scalar(
                    out=boh[:],
                    in0=iotaG_sb[:],
                    scalar1=bl_sb[:],
                    scalar2=None,
                    op0=OP.is_equal,
                )
                nc.tensor.matmul(
                    pool_ps[:, :FG],
                    lhsT=boh[:],
                    rhs=nx[:],
                    start=(w == 0),
                    stop=(w == W - 1),
                )
                nc.tensor.matmul(
                    pool_cnt[:, 0:1],
                    lhsT=boh[:],
                    rhs=ones_sb[:],
                    start=(w == 0),
                    stop=(w == W - 1),
                )
                nc.sync.dma_start(
                    out=node_x[w * 128 : (w + 1) * 128, :], in_=nx[:]
                )

            # zero tail rows (block-gather spillover for the last graphs)
            zt = pcs.tile([128, FG], bf16, tag="zt")
            nc.vector.memset(zt[:], 0.0)
            for t in range(SD // 128):
                nc.sync.dma_start(
                    out=node_x[NPC + t * 128 : NPC + (t + 1) * 128, :], in_=zt[:]
                )

            # mean = pool_sum / max(cnt, 1)
            cnt_sb = pcs.tile([GPC, 1], f32, tag="cnt")
            nc.vector.tensor_scalar(
                out=cnt_sb[:],
                in0=pool_cnt[:, 0:1],
                scalar1=1.0,
                scalar2=None,
                op0=OP.max,
            )
            rec_sb = pcs.tile([GPC, 1], f32, tag="rec")
            nc.vector.reciprocal(rec_sb[:], cnt_sb[:])
            nc.vector.tensor_scalar(
                out=mean_sb[:],
                in0=pool_ps[:, :FG],
                scalar1=rec_sb[:],
                scalar2=None,
                op0=OP.mult,
            )

        # ---------- Phase D: max pooling ----------
        # Each graph's nodes are contiguous rows of node_x; the block-gather
        # reads SD rows per graph starting at gstart[g], the mask zeroes
        # rows belonging to the next graphs (node_x >= 0 so 0 is neutral).
        max_sb = consts.tile([GPC, FG], f32, tag="maxp")
        with tc.tile_pool(name="pd_sbuf", bufs=1) as pd:
            gs_sb = pd.tile([GPC, 1], i32, tag="gs")
            nc.sync.dma_start(out=gs_sb[:], in_=gstart[:])
            dm_sb = pd.tile([GPC, SD], bf16, tag="dm")
            nc.sync.dma_start(out=dm_sb[:], in_=dmask[:])
            gat_d = pd.tile([GPC, SD * FG], bf16, tag="gat_d")
            nc.gpsimd.indirect_dma_start(
                out=gat_d[:],
                out_offset=None,
                in_=node_x[:],
                in_offset=bass.IndirectOffsetOnAxis(ap=gs_sb[:, 0:1], axis=0),
            )
            nc.vector.tensor_tensor(
                out=gat_d[:].rearrange("g (s f) -> g s f", f=FG),
                in0=gat_d[:].rearrange("g (s f) -> g s f", f=FG),
                in1=dm_sb[:, :, None].to_broadcast([GPC, SD, FG]),
                op=OP.mult,
            )
            nc.vector.reduce_max(
                out=max_sb[:],
                in_=gat_d[:].rearrange("g (s f) -> g f s", f=FG),
                axis=mybir.AxisListType.X,
            )

        # ---------- Phase E: MLP head ----------
        def linear(ctx_pool, psum_pool, act_sb, w_ap, kdim, ndim, bias_nm):
            """act_sb: [GPC, kdim] f32 sbuf -> psum tile [GPC, ndim] f32."""
            nch = (kdim + 127) // 128
            ps = psum_pool.tile([GPC, max(ndim, 2)], f32, tag="mlp_ps", space="PSUM")
            w_sb = ctx_pool.tile([128, nch * ndim], f32, tag="mlp_w")
            if nch == 1:
                nc.sync.dma_start(out=w_sb[:kdim, :ndim], in_=w_ap[:])
            else:
                nc.sync.dma_start(
                    out=w_sb[:].rearrange("p (k m) -> p k m", k=nch),
                    in_=w_ap.rearrange("(k p) m -> p k m", p=128),
                )
            nmm = nch + (1 if has_bias[bias_nm] else 0)
            for ki in range(nch):
                klo = ki * 128
                kk = min(128, kdim - klo)
                tps = psum_pool.tile([128, GPC], f32, tag="tr_ps", space="PSUM")
                nc.tensor.transpose(
                    out=tps[:kk, :GPC],
                    in_=act_sb[:, klo : klo + kk],
                    identity=ident_sb[:GPC, :GPC],
                )
                at_sb = ctx_pool.tile([128, GPC], f32, tag="at")
                nc.vector.tensor_copy(at_sb[:kk, :], tps[:kk, :])
                nc.tensor.matmul(
                    ps[:, :ndim],
                    lhsT=at_sb[:kk, :GPC],
                    rhs=w_sb[:kk, ki * ndim : (ki + 1) * ndim],
                    start=(ki == 0),
                    stop=(ki == nmm - 1),
                )
            if has_bias[bias_nm]:
                b_sb = ctx_pool.tile([1, max(ndim, 2)], f32, tag="mlp_b")
                nc.sync.dma_start(out=b_sb[:, :ndim], in_=bias_aps[bias_nm][:])
                one_row = ctx_pool.tile([1, GPC], f32, tag="onerow_e")
                nc.vector.memset(one_row[:], 1.0)
                nc.tensor.matmul(
                    ps[:, :ndim], lhsT=one_row[:], rhs=b_sb[:, :ndim],
                    start=False, stop=True,
                )
            return ps

        def elu(pe, out_sb, in_ps, n):
            r = pe.tile([GPC, n], f32, tag="elu_r")
            nc.scalar.activation(r[:], in_ps[:, :n], AF.Relu)
            m = pe.tile([GPC, n], f32, tag="elu_m")
            nc.vector.tensor_tensor(out=m[:], in0=in_ps[:, :n], in1=r[:], op=OP.subtract)
            e = pe.tile([GPC, n], f32, tag="elu_e")
            nc.scalar.activation(e[:], m[:], AF.Exp)
            nc.vector.tensor_tensor(out=e[:], in0=e[:], in1=r[:], op=OP.add)
            nc.vector.tensor_scalar(
                out=out_sb[:, :n], in0=e[:], scalar1=1.0, scalar2=None, op0=OP.subtract
            )

        with (
            tc.tile_pool(name="pe_sbuf", bufs=2) as pe,
            tc.tile_pool(name="pe_psum", bufs=2, space="PSUM") as pep,
        ):
            gx = pe.tile([GPC, 2 * FG], f32, tag="gx")
            nc.vector.tensor_copy(gx[:, :FG], mean_sb[:])
            nc.vector.tensor_copy(gx[:, FG:], max_sb[:])

            h1ps = linear(pe, pep, gx, we1, 2 * FG, HID, "be1")
            h1 = pe.tile([GPC, HID], f32, tag="h1")
            elu(pe, h1, h1ps, HID)

            h2ps = linear(pe, pep, h1, we2, HID, HID, "be2")
            h2 = pe.tile([GPC, HID], f32, tag="h2")
            nc.scalar.activation(h2[:], h2ps[:, :HID], AF.Tanh)

            mlps = linear(pe, pep, h2, we3, HID, 2 * ZD, "be3")
            mu_sb = pe.tile([GPC, ZD], f32, tag="mu")
            nc.vector.tensor_copy(mu_sb[:], mlps[:, :ZD])
            # softplus(x) = ln(1 + exp(x)); |x| is small here so this is stable
            sp_e = pe.tile([GPC, ZD], f32, tag="sp_e")
            nc.scalar.activation(sp_e[:], mlps[:, ZD : 2 * ZD], AF.Exp)
            nc.vector.tensor_scalar(
                out=sp_e[:], in0=sp_e[:], scalar1=1.0, scalar2=None, op0=OP.add
            )
            sd_sb = pe.tile([GPC, ZD], f32, tag="sd")
            nc.scalar.activation(sd_sb[:], sp_e[:], AF.Ln)
            nc.vector.tensor_scalar(
                out=sd_sb[:], in0=sd_sb[:], scalar1=1e-6, scalar2=None, op0=OP.add
            )
            eps_sb = pe.tile([GPC, ZD], f32, tag="eps")
            nc.sync.dma_start(out=eps_sb[:], in_=eps_s[:])
            z = pe.tile([GPC, ZD], f32, tag="z")
            nc.vector.tensor_tensor(out=z[:], in0=eps_sb[:], in1=sd_sb[:], op=OP.mult)
            nc.vector.tensor_tensor(out=z[:], in0=z[:], in1=mu_sb[:], op=OP.add)

            d1ps = linear(pe, pep, z, wd1, ZD, HID, "bd1")
            d1 = pe.tile([GPC, HID], f32, tag="d1")
            nc.scalar.activation(d1[:], d1ps[:, :HID], AF.Tanh)

            d2ps = linear(pe, pep, d1, wd2, HID, HID, "bd2")
            d2 = pe.tile([GPC, HID], f32, tag="d2")
            elu(pe, d2, d2ps, HID)

            yps = linear(pe, pep, d2, wd3, HID, FG, "bd3")
            y_sb = pe.tile([GPC, FG], f32, tag="ysb")
            nc.scalar.activation(y_sb[:], yps[:, :FG], AF.Sigmoid)
            nc.vector.tensor_scalar(
                out=y_sb[:],
                in0=y_sb[:],
                scalar1=1e-8,
                scalar2=1.0 - 1e-8,
                op0=OP.max,
                op1=OP.min,
            )

            nc.sync.dma_start(out=mu_out[:], in_=mu_sb[:])
            nc.sync.dma_start(out=sd_out[:], in_=sd_sb[:])
            nc.sync.dma_start(out=y_out[:], in_=y_sb[:])

        if DEBUG_DUMP:
            nc.sync.dma_start(out=d_uslab[:], in_=u_slab[:])
            nc.sync.dma_start(out=d_ufull[:], in_=u_full[:])
            nc.sync.dma_start(out=d_nodex[:], in_=node_x[:])
            nc.sync.dma_start(out=d_mean[:], in_=mean_sb[:])
            nc.sync.dma_start(out=d_max[:], in_=max_sb[:])

    nc.compile()
    _BUILD_CACHE[key] = nc
    return nc


def _install_ntff_hook():
    """Provide antenv.axon_hooks (missing in this image) so that
    run_bass_kernel_spmd(trace=True) can capture NTFF profiles via the
    axon .so's C ABI."""
    import sys, types, ctypes, contextlib

    try:
        from antenv.axon_hooks import get_axon_ntff_profile_hook  # noqa: F401

        return
    except ImportError:
        pass
    so_path = "/opt/axon/libaxon_pjrt.so"
    try:
        lib = ctypes.CDLL(so_path)
        lib.axon_start_nrt_profile.argtypes = [
            ctypes.POINTER(ctypes.c_int64),
            ctypes.c_size_t,
        ]
        lib.axon_start_nrt_profile.restype = ctypes.c_int64
        lib.axon_stop_nrt_profile.argtypes = [ctypes.c_char_p]
        lib.axon_stop_nrt_profile.restype = ctypes.c_int64
    except (OSError, AttributeError):
        lib = None

    @contextlib.contextmanager
    def _hook(output_dir, device_ids):
        import jax

        jax.devices()
        if device_ids:
            ids = (ctypes.c_int64 * len(device_ids))(*device_ids)
            rc = lib.axon_start_nrt_profile(ids, len(device_ids))
        else:
            rc = lib.axon_start_nrt_profile(None, 0)
        if rc != 0:
            raise RuntimeError(f"axon_start_nrt_profile rc={rc}")
        try:
            yield
        finally:
            n = lib.axon_stop_nrt_profile(str(output_dir).encode())
            print(f"ntff profile: {n} file(s) written to {output_dir}")

    mod = types.ModuleType("antenv.axon_hooks")
    mod.get_axon_ntff_profile_hook = lambda: (_hook if lib is not None else None)
    mod.set_axon_ntff_profile_hook = lambda h: None
    sys.modules["antenv.axon_hooks"] = mod


def kernel(**inputs):
    global LAST_RESULTS
    from concourse import bass_utils

    if TRACE:
        _install_ntff_hook()

    meta, in_maps = _preprocess(inputs)
    nc = _build(meta)
    res = bass_utils.run_bass_kernel_spmd(
        nc,
        in_maps,
        core_ids=list(range(C)),
        trace=TRACE,
        **TRACE_KWARGS,
    )
    LAST_RESULTS = res
    mu = np.concatenate([res.results[c]["mu"] for c in range(C)], axis=0)
    sd = np.concatenate([res.results[c]["stddev"] for c in range(C)], axis=0)
    y = np.concatenate([res.results[c]["y"] for c in range(C)], axis=0)
    return mu, sd, y


# revision 15
# speedup vs baseline: 1.0778x; 1.0317x over previous
"""Trainium2 Bass kernel for the GNN-VAE (GCNConv -> mean/max pool -> VAE MLPs).

Strategy (8 NeuronCores, SPMD):
  - Partition the 512 graphs into 8 groups of 64; the sorted `batch` vector
    makes each group a contiguous slab of nodes (and, after sorting edges by
    destination, a contiguous slab of edges).
  - Phase A  (per core): xw = x_slab @ Wg on the tensor engine, scaled by
    dinv -> u_slab (bf16).
  - AllGather u_slab across the 8 cores -> replicated u table (the gather
    source for message passing).
  - Phase C  (per core): per 128-node window, per 128-edge tile one indirect
    DMA gathers the messages u[src] (one row per partition).  One-hot
    matrices built on the vector engine turn the segment-sum into
    PSUM-accumulated matmuls; self-loops use a contiguous DMA + identity
    matmul.  Epilogue applies dinv[dst] + ReLU -> node_x.  Mean/count pooling
    accumulates into a persistent PSUM tile via one-hot(batch) matmuls.
  - Phase D  (per core): max pooling via one block-indirect gather (each
    graph's nodes are contiguous rows of node_x), a validity mask, and a
    segmented reduce_max.
  - Phase E  (per core): the tiny encoder/decoder MLPs on 64 graphs.
Host-side work is limited to index plumbing: sorting/partitioning edges,
degree counts, building gather index tables, and slicing inputs per core.
"""

import math
import numpy as np
import ml_dtypes

C = 8
FIN = 512
FG = 128
HID = 64
ZD = 64

BF16 = ml_dtypes.bfloat16

# module-level knobs (test.py pokes these)
DEBUG_DUMP = False
TRACE = False
TRACE_KWARGS = {}
LAST_RESULTS = None


def _preprocess(inputs):
    x = np.ascontiguousarray(np.asarray(inputs["x"], dtype=np.float32))
    ei = np.asarray(inputs["edge_index"]).astype(np.int64)
    batch = np.asarray(inputs["batch"]).astype(np.int64)
    eps = np.asarray(inputs["eps"], dtype=np.float32)

    N = x.shape[0]
    G = eps.shape[0]
    GPC = G // C
    E = ei.shape[1]

    sg = np.searchsorted(batch, np.arange(G + 1))
    core_bounds = sg[::GPC].copy()
    assert core_bounds.shape[0] == C + 1 and core_bounds[-1] == N
    ncs = np.diff(core_bounds)
    NPC = int(math.ceil(ncs.max() / 512) * 512)
    W = NPC // 128

    # degrees include the self-loop
    deg = (np.bincount(ei[1], minlength=N) + 1).astype(np.float32)
    dinv = (1.0 / np.sqrt(deg)).astype(np.float32)

    node_core = np.searchsorted(core_bounds, np.arange(N), side="right") - 1
    pid = (node_core * NPC + (np.arange(N) - core_bounds[node_core])).astype(np.int64)

    # real edges only, sorted by destination (self-loops handled separately)
    order = np.argsort(ei[1], kind="stable")
    dsts = ei[1][order]
    srcs_pid = pid[ei[0][order]].astype(np.int32)
    core_edge_bounds = np.searchsorted(dsts, core_bounds)

    dst_core_all = np.searchsorted(core_bounds, dsts, side="right") - 1
    dst_loc_all = dsts - core_bounds[dst_core_all]
    cw = dst_core_all * W + (dst_loc_all >> 7)
    cnts = np.bincount(cw, minlength=C * W)
    S = int(math.ceil(cnts.max() / 128))

    gsz = np.diff(sg)
    Lmax = int(gsz.max())
    SD = int(math.ceil(max(Lmax, 1) / 128) * 128)

    idx_arr = np.zeros((C, W, 128, S), dtype=np.int32)
    dslot_arr = np.full((C, W, 128, S), -1.0, dtype=np.float32)
    blocal_arr = np.full((C, W * 128), -1.0, dtype=np.float32)
    dinv_arr = np.zeros((C, W * 128), dtype=np.float32)
    xt_arr = np.zeros((C, W, 128, FIN // 128, 128), dtype=np.float32)
    gstart_arr = np.zeros((C, GPC, 1), dtype=np.int32)
    dmask_arr = np.zeros((C, GPC, SD), dtype=np.float32)
    eps_arr = np.zeros((C, GPC, ZD), dtype=np.float32)

    for c in range(C):
        lo, hi = core_bounds[c], core_bounds[c + 1]
        n_c = hi - lo
        e0, e1 = core_edge_bounds[c], core_edge_bounds[c + 1]
        dloc = (dsts[e0:e1] - lo).astype(np.int64)
        spid = srcs_pid[e0:e1]
        win = dloc >> 7
        slot = (dloc & 127).astype(np.float32)
        starts = np.searchsorted(win, np.arange(W))
        rank = np.arange(e1 - e0) - starts[win]
        p = rank % 128
        j = rank // 128
        idx_arr[c, win, p, j] = spid
        dslot_arr[c, win, p, j] = slot

        blocal_arr[c, :n_c] = batch[lo:hi] - c * GPC
        dinv_arr[c, :n_c] = dinv[lo:hi]

        xs = np.zeros((NPC, FIN), dtype=np.float32)
        xs[:n_c] = x[lo:hi]
        # xt[w, p, k, m] = xs[w*128 + m, k*128 + p]
        xt_arr[c] = xs.reshape(W, 128, FIN // 128, 128).transpose(0, 3, 2, 1)

        for g in range(GPC):
            s = sg[c * GPC + g] - lo
            L = gsz[c * GPC + g]
            gstart_arr[c, g, 0] = s
            dmask_arr[c, g, :L] = 1.0
        eps_arr[c] = eps[c * GPC : (c + 1) * GPC]

    iota128 = np.tile(np.arange(128, dtype=np.float32), (128, 1)).astype(BF16)
    iotaG = np.tile(np.arange(GPC, dtype=np.float32), (128, 1)).astype(BF16)
    ones_col = np.ones((128, 1), dtype=BF16)
    ident64 = np.eye(64, dtype=np.float32)
    ident128b = np.eye(128, dtype=np.float32).astype(BF16)

    weights = {}
    for nm in ("Wg", "We1", "We2", "We3", "Wd1", "Wd2", "Wd3"):
        weights[nm] = np.ascontiguousarray(np.asarray(inputs[nm], dtype=np.float32))
    biases = {}
    for nm in ("bg", "be1", "be2", "be3", "bd1", "bd2", "bd3"):
        biases[nm] = np.asarray(inputs[nm], dtype=np.float32).reshape(1, -1)
    has_bias = {nm: bool(np.any(b != 0.0)) for nm, b in biases.items()}

    # pack per-4-window chunks so DMA descriptors are 4x bigger
    assert W % 4 == 0, W
    W4 = W // 4
    idx4 = (idx_arr.reshape(C, W4, 4, 128, S).transpose(0, 1, 3, 2, 4)
            .reshape(C, W4, 128, 4 * S).copy())
    dslot4 = (dslot_arr.reshape(C, W4, 4, 128, S).transpose(0, 1, 3, 2, 4)
              .reshape(C, W4, 128, 4 * S).copy())
    xt4 = (xt_arr.reshape(C, W4, 4, 128, (FIN // 128) * 128)
           .transpose(0, 1, 3, 2, 4).reshape(C, W4, 128, 4 * FIN).copy())
    dinv_pw = dinv_arr.reshape(C, W, 128).transpose(0, 2, 1).copy()
    blocal_pw = blocal_arr.reshape(C, W, 128).transpose(0, 2, 1).copy()

    meta = dict(N=N, G=G, GPC=GPC, E=E, NPC=NPC, W=W, S=S, SD=SD, has_bias=has_bias)

    in_maps = []
    for c in range(C):
        m = dict(
            xt=xt4[c].astype(BF16),
            idx=idx4[c],
            dslot=dslot4[c],
            blocal=blocal_pw[c],
            dinvw=dinv_pw[c],
            gstart=gstart_arr[c],
            dmask=dmask_arr[c].astype(BF16),
            eps_s=eps_arr[c],
            iota128=iota128,
            iotaG=iotaG,
            ones_col=ones_col,
            ident64=ident64,
            ident128b=ident128b,
        )
        for nm, wv in weights.items():
            m[nm] = wv
        for nm, bv in biases.items():
            if has_bias[nm]:
                m[nm] = bv
        in_maps.append(m)
    return meta, in_maps


_BUILD_CACHE = {}


def _build(meta):
    key = (meta["NPC"], meta["S"], meta["SD"], DEBUG_DUMP,
           tuple(sorted(meta["has_bias"].items())))
    if key in _BUILD_CACHE:
        return _BUILD_CACHE[key]

    from concourse import bass, bacc, tile, mybir
    from contextlib import ExitStack

    NPC, W, S, GPC = meta["NPC"], meta["W"], meta["S"], meta["GPC"]
    SD = meta["SD"]
    has_bias = meta["has_bias"]
    KC = FIN // 128  # k chunks for the input matmul

    f32 = mybir.dt.float32
    bf16 = mybir.dt.bfloat16
    i32 = mybir.dt.int32
    AF = mybir.ActivationFunctionType
    OP = mybir.AluOpType

    nc = bacc.Bacc(
        "TRN2",
        target_bir_lowering=False,
        debug=False,
        enable_asserts=False,
        num_devices=C,
    )

    # ---- I/O ----
    W4 = W // 4
    xt = nc.dram_tensor("xt", [W4, 128, 4 * FIN], bf16, kind="ExternalInput").ap()
    idx = nc.dram_tensor("idx", [W4, 128, 4 * S], i32, kind="ExternalInput").ap()
    dslot = nc.dram_tensor("dslot", [W4, 128, 4 * S], f32, kind="ExternalInput").ap()
    blocal = nc.dram_tensor("blocal", [128, W], f32, kind="ExternalInput").ap()
    dinvw = nc.dram_tensor("dinvw", [128, W], f32, kind="ExternalInput").ap()
    gstart = nc.dram_tensor("gstart", [GPC, 1], i32, kind="ExternalInput").ap()
    dmask = nc.dram_tensor("dmask", [GPC, SD], bf16, kind="ExternalInput").ap()
    eps_s = nc.dram_tensor("eps_s", [GPC, ZD], f32, kind="ExternalInput").ap()
    iota128 = nc.dram_tensor("iota128", [128, 128], bf16, kind="ExternalInput").ap()
    iotaG = nc.dram_tensor("iotaG", [128, GPC], bf16, kind="ExternalInput").ap()
    ones_col = nc.dram_tensor("ones_col", [128, 1], bf16, kind="ExternalInput").ap()
    ident64 = nc.dram_tensor("ident64", [64, 64], f32, kind="ExternalInput").ap()
    ident128b = nc.dram_tensor("ident128b", [128, 128], bf16, kind="ExternalInput").ap()
    wg = nc.dram_tensor("Wg", [FIN, FG], f32, kind="ExternalInput").ap()
    we1 = nc.dram_tensor("We1", [2 * FG, HID], f32, kind="ExternalInput").ap()
    we2 = nc.dram_tensor("We2", [HID, HID], f32, kind="ExternalInput").ap()
    we3 = nc.dram_tensor("We3", [HID, 2 * ZD], f32, kind="ExternalInput").ap()
    wd1 = nc.dram_tensor("Wd1", [ZD, HID], f32, kind="ExternalInput").ap()
    wd2 = nc.dram_tensor("Wd2", [HID, HID], f32, kind="ExternalInput").ap()
    wd3 = nc.dram_tensor("Wd3", [HID, FG], f32, kind="ExternalInput").ap()
    bias_aps = {}
    bias_shapes = dict(bg=FG, be1=HID, be2=HID, be3=2 * ZD, bd1=HID, bd2=HID, bd3=FG)
    for nm, n in bias_shapes.items():
        if has_bias[nm]:
            bias_aps[nm] = nc.dram_tensor(nm, [1, n], f32, kind="ExternalInput").ap()

    if DEBUG_DUMP:
        d_uslab = nc.dram_tensor("d_uslab", [NPC, FG], bf16, kind="ExternalOutput").ap()
        d_ufull = nc.dram_tensor("d_ufull", [C * NPC, FG], bf16, kind="ExternalOutput").ap()
        d_nodex = nc.dram_tensor("d_nodex", [NPC + SD, FG], bf16, kind="ExternalOutput").ap()
        d_mean = nc.dram_tensor("d_mean", [GPC, FG], f32, kind="ExternalOutput").ap()
        d_max = nc.dram_tensor("d_max", [GPC, FG], f32, kind="ExternalOutput").ap()
    mu_out = nc.dram_tensor("mu", [GPC, ZD], f32, kind="ExternalOutput").ap()
    sd_out = nc.dram_tensor("stddev", [GPC, ZD], f32, kind="ExternalOutput").ap()
    y_out = nc.dram_tensor("y", [GPC, FG], f32, kind="ExternalOutput").ap()

    # ---- internal DRAM ----
    u_slab = nc.dram_tensor("u_slab", [NPC, FG], bf16).ap()
    u_full = nc.dram_tensor("u_full", [C * NPC, FG], bf16, addr_space="Shared").ap()
    node_x = nc.dram_tensor("node_x", [NPC + SD, FG], bf16).ap()

    with tile.TileContext(nc) as tc, ExitStack() as ctx:
        consts = ctx.enter_context(tc.tile_pool(name="consts", bufs=1))

        wg_sb = consts.tile([128, KC * FG], bf16, tag="wg")
        nc.gpsimd.dma_start(
            out=wg_sb[:].rearrange("p (k m) -> p k m", k=KC),
            in_=wg.rearrange("(k p) m -> p k m", p=128),
        )
        iota128_sb = consts.tile([128, 128], bf16, tag="iota128")
        nc.sync.dma_start(out=iota128_sb[:], in_=iota128[:])
        iotaG_sb = consts.tile([128, GPC], bf16, tag="iotaG")
        nc.sync.dma_start(out=iotaG_sb[:], in_=iotaG[:])
        ones_sb = consts.tile([128, 1], bf16, tag="ones")
        nc.sync.dma_start(out=ones_sb[:], in_=ones_col[:])
        ident_sb = consts.tile([64, 64], f32, tag="ident")
        nc.sync.dma_start(out=ident_sb[:], in_=ident64[:])
        identb_sb = consts.tile([128, 128], bf16, tag="identb")
        nc.sync.dma_start(out=identb_sb[:], in_=ident128b[:])
        dinv_sb = consts.tile([128, W], f32, tag="dinvpw")
        nc.sync.dma_start(out=dinv_sb[:], in_=dinvw[:])
        bloc_sb = consts.tile([128, W], f32, tag="blocpw")
        nc.sync.dma_start(out=bloc_sb[:], in_=blocal[:])

        # ---------- Phase A: u_slab = dinv * (x @ Wg) ----------
        with (
            tc.tile_pool(name="pa_sbuf", bufs=3) as pa,
            tc.tile_pool(name="pa_psum", bufs=2, space="PSUM") as pap,
        ):
            for w4 in range(W4):
                xt_sb = pa.tile([128, 4 * FIN], bf16, tag="xt")
                nc.sync.dma_start(out=xt_sb[:], in_=xt[w4])
                for f in range(4):
                    w = 4 * w4 + f
                    ps = pap.tile([128, FG], f32, tag="pa_ps", space="PSUM")
                    for k in range(KC):
                        nc.tensor.matmul(
                            ps[:],
                            lhsT=xt_sb[:, (f * KC + k) * 128 : (f * KC + k + 1) * 128],
                            rhs=wg_sb[:, k * FG : (k + 1) * FG],
                            start=(k == 0),
                            stop=(k == KC - 1),
                        )
                    u_sb = pa.tile([128, FG], bf16, tag="u")
                    nc.scalar.activation(
                        u_sb[:], ps[:], AF.Copy, scale=dinv_sb[:, w : w + 1]
                    )
                    nc.sync.dma_start(
                        out=u_slab[w * 128 : (w + 1) * 128, :], in_=u_sb[:]
                    )

        # ---------- AllGather ----------
        nc.gpsimd.collective_compute(
            "AllGather",
            mybir.AluOpType.bypass,
            replica_groups=[list(range(C))],
            ins=[u_slab[:]],
            outs=[u_full[:]],
        )

        # ---------- Phase C: message passing + mean pooling ----------
        pool_ps_pool = ctx.enter_context(
            tc.tile_pool(name="pool_ps", bufs=1, space="PSUM")
        )
        pool_ps = pool_ps_pool.tile([GPC, FG], f32, tag="pool", space="PSUM")
        pool_cnt = pool_ps_pool.tile([GPC, 2], f32, tag="poolcnt", space="PSUM")
        mean_sb = consts.tile([GPC, FG], f32, tag="mean")
        with (
            tc.tile_pool(name="pc_sbuf", bufs=4) as pcs,
            tc.tile_pool(name="pc_meta", bufs=12) as pcm,
            tc.tile_pool(name="pc_psum", bufs=2, space="PSUM") as pcp,
        ):
            # zero tail rows first (no dependence on the windows)
            zt = pcs.tile([128, FG], bf16, tag="zt")
            nc.vector.memset(zt[:], 0.0)
            for t in range(SD // 128):
                nc.sync.dma_start(
                    out=node_x[NPC + t * 128 : NPC + (t + 1) * 128, :], in_=zt[:]
                )

            for w in range(W):
                f = w % 4
                if f == 0:
                    idx_sb = pcm.tile([128, 4 * S], i32, tag="idx")
                    nc.sync.dma_start(out=idx_sb[:], in_=idx[w // 4])
                    ds_sb = pcm.tile([128, 4 * S], f32, tag="ds")
                    nc.sync.dma_start(out=ds_sb[:], in_=dslot[w // 4])

                gat = pcs.tile([128, (S + 1) * FG], bf16, tag="gat")
                for j in range(S):
                    nc.gpsimd.indirect_dma_start(
                        out=gat[:, j * FG : (j + 1) * FG],
                        out_offset=None,
                        in_=u_full[:],
                        in_offset=bass.IndirectOffsetOnAxis(
                            ap=idx_sb[:, f * S + j : f * S + j + 1], axis=0
                        ),
                    )
                # self-loop messages: own window's u rows, contiguous
                nc.sync.dma_start(
                    out=gat[:, S * FG : (S + 1) * FG],
                    in_=u_slab[w * 128 : (w + 1) * 128, :],
                )

                oh = pcs.tile([128, S * 128], bf16, tag="oh")
                for j in range(S):
                    nc.vector.tensor_scalar(
                        out=oh[:, j * 128 : (j + 1) * 128],
                        in0=iota128_sb[:],
                        scalar1=ds_sb[:, f * S + j : f * S + j + 1],
                        scalar2=None,
                        op0=OP.is_equal,
                    )

                wps = pcp.tile([128, FG], f32, tag="wps", space="PSUM")
                nmm = S + 1 + (1 if has_bias["bg"] else 0)
                for j in range(S):
                    nc.tensor.matmul(
                        wps[:],
                        lhsT=oh[:, j * 128 : (j + 1) * 128],
                        rhs=gat[:, j * FG : (j + 1) * FG],
                        start=(j == 0),
                        stop=False,
                    )
                nc.tensor.matmul(
                    wps[:],
                    lhsT=identb_sb[:],
                    rhs=gat[:, S * FG : (S + 1) * FG],
                    start=False,
                    stop=(nmm == S + 1),
                )
                if has_bias["bg"]:
                    bg_sb = pcm.tile([1, FG], f32, tag="bgrow")
                    nc.sync.dma_start(out=bg_sb[:], in_=bias_aps["bg"][:])
                    one_row = pcm.tile([1, 128], f32, tag="onerow")
                    nc.vector.memset(one_row[:], 1.0)
                    nc.tensor.matmul(
                        wps[:], lhsT=one_row[:], rhs=bg_sb[:], start=False, stop=True
                    )

                nx = pcs.tile([128, FG], bf16, tag="nx")
                nc.scalar.activation(nx[:], wps[:], AF.Relu, scale=dinv_sb[:, w : w + 1])

                boh = pcs.tile([128, GPC], bf16, tag="boh")
                nc.vector.tensor_scalar(
                    out=boh[:],
                    in0=iotaG_sb[:],
                    scalar1=bloc_sb[:, w : w + 1],
                    scalar2=None,
                    op0=OP.is_equal,
                )
                nc.tensor.matmul(
                    pool_ps[:, :FG],
                    lhsT=boh[:],
                    rhs=nx[:],
                    start=(w == 0),
                    stop=(w == W - 1),
                )
                nc.tensor.matmul(
                    pool_cnt[:, 0:1],
                    lhsT=boh[:],
                    rhs=ones_sb[:],
                    start=(w == 0),
                    stop=(w == W - 1),
                )
                nc.sync.dma_start(
                    out=node_x[w * 128 : (w + 1) * 128, :], in_=nx[:]
                )

            # mean = pool_sum / max(cnt, 1)
            cnt_sb = pcs.tile([GPC, 1], f32, tag="cnt")
            nc.vector.tensor_scalar(
                out=cnt_sb[:],
                in0=pool_cnt[:, 0:1],
                scalar1=1.0,
                scalar2=None,
                op0=OP.max,
            )
            rec_sb = pcs.tile([GPC, 1], f32, tag="rec")
            nc.vector.reciprocal(rec_sb[:], cnt_sb[:])
            nc.vector.tensor_scalar(
                out=mean_sb[:],
                in0=pool_ps[:, :FG],
                scalar1=rec_sb[:],
                scalar2=None,
                op0=OP.mult,
            )

        # ---------- Phase D: max pooling ----------
        # Each graph's nodes are contiguous rows of node_x; the block-gather
        # reads SD rows per graph starting at gstart[g], the mask zeroes
        # rows belonging to the next graphs (node_x >= 0 so 0 is neutral).
        max_sb = consts.tile([GPC, FG], f32, tag="maxp")
        with tc.tile_pool(name="pd_sbuf", bufs=1) as pd:
            gs_sb = pd.tile([GPC, 1], i32, tag="gs")
            nc.sync.dma_start(out=gs_sb[:], in_=gstart[:])
            dm_sb = pd.tile([GPC, SD], bf16, tag="dm")
            nc.sync.dma_start(out=dm_sb[:], in_=dmask[:])
            gat_d = pd.tile([GPC, SD * FG], bf16, tag="gat_d")
            nc.gpsimd.indirect_dma_start(
                out=gat_d[:],
                out_offset=None,
                in_=node_x[:],
                in_offset=bass.IndirectOffsetOnAxis(ap=gs_sb[:, 0:1], axis=0),
            )
            nc.vector.tensor_tensor(
                out=gat_d[:].rearrange("g (s f) -> g s f", f=FG),
                in0=gat_d[:].rearrange("g (s f) -> g s f", f=FG),
                in1=dm_sb[:, :, None].to_broadcast([GPC, SD, FG]),
                op=OP.mult,
            )
            nc.vector.reduce_max(
                out=max_sb[:],
                in_=gat_d[:].rearrange("g (s f) -> g f s", f=FG),
                axis=mybir.AxisListType.X,
            )

        # ---------- Phase E: MLP head ----------
        def linear(ctx_pool, psum_pool, act_sb, w_ap, kdim, ndim, bias_nm):
            """act_sb: [GPC, kdim] f32 sbuf -> psum tile [GPC, ndim] f32."""
            nch = (kdim + 127) // 128
            ps = psum_pool.tile([GPC, max(ndim, 2)], f32, tag="mlp_ps", space="PSUM")
            w_sb = ctx_pool.tile([128, nch * ndim], f32, tag="mlp_w")
            if nch == 1:
                nc.sync.dma_start(out=w_sb[:kdim, :ndim], in_=w_ap[:])
            else:
                nc.sync.dma_start(
                    out=w_sb[:].rearrange("p (k m) -> p k m", k=nch),
                    in_=w_ap.rearrange("(k p) m -> p k m", p=128),
                )
            nmm = nch + (1 if has_bias[bias_nm] else 0)
            for ki in range(nch):
                klo = ki * 128
                kk = min(128, kdim - klo)
                tps = psum_pool.tile([128, GPC], f32, tag="tr_ps", space="PSUM")
                nc.tensor.transpose(
                    out=tps[:kk, :GPC],
                    in_=act_sb[:, klo : klo + kk],
                    identity=ident_sb[:GPC, :GPC],
                )
                at_sb = ctx_pool.tile([128, GPC], f32, tag="at")
                nc.vector.tensor_copy(at_sb[:kk, :], tps[:kk, :])
                nc.tensor.matmul(
                    ps[:, :ndim],
                    lhsT=at_sb[:kk, :GPC],
                    rhs=w_sb[:kk, ki * ndim : (ki + 1) * ndim],
                    start=(ki == 0),
                    stop=(ki == nmm - 1),
                )
            if has_bias[bias_nm]:
                b_sb = ctx_pool.tile([1, max(ndim, 2)], f32, tag="mlp_b")
                nc.sync.dma_start(out=b_sb[:, :ndim], in_=bias_aps[bias_nm][:])
                one_row = ctx_pool.tile([1, GPC], f32, tag="onerow_e")
                nc.vector.memset(one_row[:], 1.0)
                nc.tensor.matmul(
                    ps[:, :ndim], lhsT=one_row[:], rhs=b_sb[:, :ndim],
                    start=False, stop=True,
                )
            return ps

        def elu(pe, out_sb, in_ps, n):
            r = pe.tile([GPC, n], f32, tag="elu_r")
            nc.scalar.activation(r[:], in_ps[:, :n], AF.Relu)
            m = pe.tile([GPC, n], f32, tag="elu_m")
            nc.vector.tensor_tensor(out=m[:], in0=in_ps[:, :n], in1=r[:], op=OP.subtract)
            e = pe.tile([GPC, n], f32, tag="elu_e")
            nc.scalar.activation(e[:], m[:], AF.Exp)
            nc.vector.tensor_tensor(out=e[:], in0=e[:], in1=r[:], op=OP.add)
            nc.vector.tensor_scalar(
                out=out_sb[:, :n], in0=e[:], scalar1=1.0, scalar2=None, op0=OP.subtract
            )

        with (
            tc.tile_pool(name="pe_sbuf", bufs=2) as pe,
            tc.tile_pool(name="pe_psum", bufs=2, space="PSUM") as pep,
        ):
            gx = pe.tile([GPC, 2 * FG], f32, tag="gx")
            nc.vector.tensor_copy(gx[:, :FG], mean_sb[:])
            nc.vector.tensor_copy(gx[:, FG:], max_sb[:])

            h1ps = linear(pe, pep, gx, we1, 2 * FG, HID, "be1")
            h1 = pe.tile([GPC, HID], f32, tag="h1")
            elu(pe, h1, h1ps, HID)

            h2ps = linear(pe, pep, h1, we2, HID, HID, "be2")
            h2 = pe.tile([GPC, HID], f32, tag="h2")
            nc.scalar.activation(h2[:], h2ps[:, :HID], AF.Tanh)

            mlps = linear(pe, pep, h2, we3, HID, 2 * ZD, "be3")
            mu_sb = pe.tile([GPC, ZD], f32, tag="mu")
            nc.vector.tensor_copy(mu_sb[:], mlps[:, :ZD])
            # softplus(x) = ln(1 + exp(x)); |x| is small here so this is stable
            sp_e = pe.tile([GPC, ZD], f32, tag="sp_e")
            nc.scalar.activation(sp_e[:], mlps[:, ZD : 2 * ZD], AF.Exp)
            nc.vector.tensor_scalar(
                out=sp_e[:], in0=sp_e[:], scalar1=1.0, scalar2=None, op0=OP.add
            )
            sd_sb = pe.tile([GPC, ZD], f32, tag="sd")
            nc.scalar.activation(sd_sb[:], sp_e[:], AF.Ln)
            nc.vector.tensor_scalar(
                out=sd_sb[:], in0=sd_sb[:], scalar1=1e-6, scalar2=None, op0=OP.add
            )
            eps_sb = pe.tile([GPC, ZD], f32, tag="eps")
            nc.sync.dma_start(out=eps_sb[:], in_=eps_s[:])
            z = pe.tile([GPC, ZD], f32, tag="z")
            nc.vector.tensor_tensor(out=z[:], in0=eps_sb[:], in1=sd_sb[:], op=OP.mult)
            nc.vector.tensor_tensor(out=z[:], in0=z[:], in1=mu_sb[:], op=OP.add)

            d1ps = linear(pe, pep, z, wd1, ZD, HID, "bd1")
            d1 = pe.tile([GPC, HID], f32, tag="d1")
            nc.scalar.activation(d1[:], d1ps[:, :HID], AF.Tanh)

            d2ps = linear(pe, pep, d1, wd2, HID, HID, "bd2")
            d2 = pe.tile([GPC, HID], f32, tag="d2")
            elu(pe, d2, d2ps, HID)

            yps = linear(pe, pep, d2, wd3, HID, FG, "bd3")
            y_sb = pe.tile([GPC, FG], f32, tag="ysb")
            nc.scalar.activation(y_sb[:], yps[:, :FG], AF.Sigmoid)
            nc.vector.tensor_scalar(
                out=y_sb[:],
                in0=y_sb[:],
                scalar1=1e-8,
                scalar2=1.0 - 1e-8,
                op0=OP.max,
                op1=OP.min,
            )

            nc.sync.dma_start(out=mu_out[:], in_=mu_sb[:])
            nc.sync.dma_start(out=sd_out[:], in_=sd_sb[:])
            nc.sync.dma_start(out=y_out[:], in_=y_sb[:])

        if DEBUG_DUMP:
            nc.sync.dma_start(out=d_uslab[:], in_=u_slab[:])
            nc.sync.dma_start(out=d_ufull[:], in_=u_full[:])
            nc.sync.dma_start(out=d_nodex[:], in_=node_x[:])
            nc.sync.dma_start(out=d_mean[:], in_=mean_sb[:])
            nc.sync.dma_start(out=d_max[:], in_=max_sb[:])

    nc.compile()
    _BUILD_CACHE[key] = nc
    return nc


def _install_ntff_hook():
    """Provide antenv.axon_hooks (missing in this image) so that
    run_bass_kernel_spmd(trace=True) can capture NTFF profiles via the
    axon .so's C ABI."""
    import sys, types, ctypes, contextlib

    try:
        from antenv.axon_hooks import get_axon_ntff_profile_hook  # noqa: F401

        return
    except ImportError:
        pass
    so_path = "/opt/axon/libaxon_pjrt.so"
    try:
        lib = ctypes.CDLL(so_path)
        lib.axon_start_nrt_profile.argtypes = [
            ctypes.POINTER(ctypes.c_int64),
            ctypes.c_size_t,
        ]
        lib.axon_start_nrt_profile.restype = ctypes.c_int64
        lib.axon_stop_nrt_profile.argtypes = [ctypes.c_char_p]
        lib.axon_stop_nrt_profile.restype = ctypes.c_int64
    except (OSError, AttributeError):
        lib = None

    @contextlib.contextmanager
    def _hook(output_dir, device_ids):
        import jax

        jax.devices()
        if device_ids:
            ids = (ctypes.c_int64 * len(device_ids))(*device_ids)
            rc = lib.axon_start_nrt_profile(ids, len(device_ids))
        else:
            rc = lib.axon_start_nrt_profile(None, 0)
        if rc != 0:
            raise RuntimeError(f"axon_start_nrt_profile rc={rc}")
        try:
            yield
        finally:
            n = lib.axon_stop_nrt_profile(str(output_dir).encode())
            print(f"ntff profile: {n} file(s) written to {output_dir}")

    mod = types.ModuleType("antenv.axon_hooks")
    mod.get_axon_ntff_profile_hook = lambda: (_hook if lib is not None else None)
    mod.set_axon_ntff_profile_hook = lambda h: None
    sys.modules["antenv.axon_hooks"] = mod


def kernel(**inputs):
    global LAST_RESULTS
    from concourse import bass_utils

    if TRACE:
        _install_ntff_hook()

    meta, in_maps = _preprocess(inputs)
    nc = _build(meta)
    res = bass_utils.run_bass_kernel_spmd(
        nc,
        in_maps,
        core_ids=list(range(C)),
        trace=TRACE,
        **TRACE_KWARGS,
    )
    LAST_RESULTS = res
    mu = np.concatenate([res.results[c]["mu"] for c in range(C)], axis=0)
    sd = np.concatenate([res.results[c]["stddev"] for c in range(C)], axis=0)
    y = np.concatenate([res.results[c]["y"] for c in range(C)], axis=0)
    return mu, sd, y
